# revision 9
# baseline (speedup 1.0000x reference)
"""HalfKP NNUE feature-transformer + MLP head for 8 Trainium2 NeuronCores.

Strategy (data-parallel over batch, fp8 feature stream):
  - Each of the 8 cores gets B/8 = 1024 batch rows of white/black features.
  - Features are centered+scaled on host: g = (f - 0.5) * 16, quantized to
    fp8 e3m4 (1 byte/elem -> 4x less HBM traffic than fp32). The 0.5 offset
    folds into the ft bias: w.f = (w.g)/16 + 0.5*sum(w) + ft_b.
  - ft weights stay bf16 (PE supports mixed bf16 lhsT x fp8 rhs) -> weight
    quantization error is negligible; only the e3m4 feature error remains
    (max rel err ~6e-3 on the final output).
  - PE 4-way column tiling: col group g = (side, half) accumulates its own
    [4, 512] slice of one PSUM bank at partitions 32g..32g+4. The four
    matmuls per k-tile run concurrently in the 128x128 array, so PE time is
    well below the DMA time -> kernel is HBM-bandwidth bound (~414 GB/s
    burst on 16 SDMA engines).
  - Host pre-transposes each feature shard to [K, Bc] so the contraction dim
    (K = 40960) lands on SBUF partitions with fully contiguous 16 KB/
    partition DMA descriptors. White chunks issue on the SP HWDGE ring,
    black chunks on the ACT ring, with separate sems so white's matmuls and
    the white-side tail math run while black still streams.
  - Tail (stm blend + clips + l1/l2) runs on [<=8, 1024] tiles; the tiny
    l1/l2 matmuls use fp32r (single-pass PE) to shorten the serial tail.
"""

import numpy as np
import ml_dtypes

import concourse.bass as bass
import concourse.bacc as bacc_mod
import concourse.mybir as mybir
from concourse.tile import TileContext
from concourse.bass_utils import run_bass_kernel_spmd

N_CORES = 8
B = 8192
K = 40960
M = 4
BC = B // N_CORES        # 1024 batch rows per core
CHUNK = 2048             # feature (k) rows per DMA chunk
J = CHUNK // 128         # k-slices per chunk (16)
NCHUNK = K // CHUNK      # 20
NB = BC // 512           # halves (matmul free-dim limit is 512 fp32)
NT = K // 128            # total k-tiles (320)
HB = J * BC              # free extent per chunk (8192)

FSCALE = 16.0            # feature scale: g = (f - 0.5) * FSCALE
FEAT_BUFS = 4

E3 = ml_dtypes.float8_e3m4
BF = ml_dtypes.bfloat16

_nc_cache = {}


def _build_nc():
    key = (CHUNK, FEAT_BUFS)
    if key in _nc_cache:
        return _nc_cache[key]
    f32 = mybir.dt.float32
    f32r = mybir.dt.float32r
    bf16 = mybir.dt.bfloat16
    e3 = mybir.dt.float8e3
    alu = mybir.AluOpType
    nc = bacc_mod.Bacc(trn_type="TRN2")

    featw = nc.dram_tensor("featw", [NCHUNK, 128, HB], e3,
                           kind="ExternalInput")
    featb = nc.dram_tensor("featb", [NCHUNK, 128, HB], e3,
                           kind="ExternalInput")
    # weight pack: wsb[p, 4t+m] = ft_w[m, k(t,p)] (bf16)
    wsb = nc.dram_tensor("wsb", [128, NT * M], bf16, kind="ExternalInput")
    consts = nc.dram_tensor("consts", [8, 20], f32, kind="ExternalInput")
    stm4 = nc.dram_tensor("stm4", [M, BC], f32, kind="ExternalInput")
    out = nc.dram_tensor("out", [1, BC], f32, kind="ExternalOutput")

    with TileContext(nc) as tc:
        with (
            tc.tile_pool(name="const", bufs=1) as cpool,
            tc.tile_pool(name="feat", bufs=FEAT_BUFS) as fpool,
            tc.tile_pool(name="psum", bufs=1, space="PSUM") as ppool,
            tc.tile_pool(name="tail", bufs=1) as tpool,
        ):
            # weights/consts lead the ACT ring (tiny, unblock the PE
            # warmups fast); feature chunks follow: white on SP, black on ACT.
            w_tile = cpool.tile([128, NT * M], bf16, tag="w")
            nc.scalar.dma_start(out=w_tile[:], in_=wsb[:])
            c_tile = cpool.tile([8, 20], f32, tag="c")
            nc.scalar.dma_start(out=c_tile[:], in_=consts[:])
            s_tile = cpool.tile([M, BC], f32, tag="s")
            nc.scalar.dma_start(out=s_tile[:], in_=stm4[:])

            # segments: (chunk, j0, jn) — one full chunk each
            segs = [(c, 0, J) for c in range(NCHUNK)]

            def seg_dma(i):
                c, j0, jn = segs[i]
                sub = jn != J
                tg = "l" if sub else ""
                ftw = fpool.tile([128, jn * BC], e3, tag=f"fw{tg}",
                                 name=f"fw{c}_{j0}")
                ftb_ = fpool.tile([128, jn * BC], e3, tag=f"fb{tg}",
                                  name=f"fb{c}_{j0}")
                # alternate rings per segment to balance the two HWDGE queues
                ring_w = nc.sync if i % 2 == 0 else nc.scalar
                ring_b = nc.scalar if i % 2 == 0 else nc.sync
                ring_w.dma_start(out=ftw[:], in_=featw[c, :, j0 * BC:(j0 + jn) * BC])
                ring_b.dma_start(out=ftb_[:], in_=featb[c, :, j0 * BC:(j0 + jn) * BC])
                return ftw, ftb_

            fts = [seg_dma(i) for i in range(FEAT_BUFS - 1)]

            # accumulator bank: col group g=(side,half) owns ps[32g:32g+4]
            ps = ppool.tile([128, 512], f32, tag="acc", name="acc")
            p1 = ppool.tile([8, BC], f32, tag="p1")
            # Warmup matmuls: consume the w_tile/c_tile DMA deps on PE so no
            # later matmul needs two sem waits (one HW wait slot per inst).
            nc.tensor.matmul(ps[0:4, 0:4], w_tile[:, 0:4], w_tile[:, 0:4],
                             start=True, stop=True, skip_group_check=True)
            nc.tensor.matmul(p1[0:8, 0:8], c_tile[0:4, 0:8],
                             c_tile[0:4, 0:8], start=True, stop=True,
                             skip_group_check=True)

            for i in range(len(segs)):
                c, j0, jn = segs[i]
                ftw, ftb_ = fts[i] if i < FEAT_BUFS - 1 else seg_dma(i)
                for j in range(j0, j0 + jn):
                    t = c * J + j
                    wt = w_tile[:, M * t: M * (t + 1)]
                    for g in range(4):
                        s, h = g >> 1, g & 1
                        src = ftw if s == 0 else ftb_
                        off = (j - j0) * BC + h * 512
                        nc.tensor.matmul(
                            ps[32 * g: 32 * g + 4, :],
                            wt, src[:, off: off + 512],
                            start=(t == 0),
                            stop=(t == NT - 1),
                            tile_position=(0, 32 * g),
                            skip_group_check=True,
                        )

            # ---- tail: scale+bias, stm blend, clips, l1, l2 ----
            # white-side combine depends only on col groups 0/1 -> runs
            # while black still streams.
            ftb = c_tile[0:M, 17:18]
            sw = tpool.tile([M, BC], f32, tag="sw")
            sb = tpool.tile([M, BC], f32, tag="sb")
            for h in range(NB):
                sl = slice(h * 512, (h + 1) * 512)
                nc.vector.tensor_scalar(
                    out=sw[:, sl], in0=ps[32 * h: 32 * h + 4, :],
                    scalar1=1.0 / FSCALE, scalar2=ftb,
                    op0=alu.mult, op1=alu.add)
            for h in range(NB):
                sl = slice(h * 512, (h + 1) * 512)
                nc.vector.tensor_scalar(
                    out=sb[:, sl], in0=ps[64 + 32 * h: 68 + 32 * h, :],
                    scalar1=1.0 / FSCALE, scalar2=ftb,
                    op0=alu.mult, op1=alu.add)
            c_r = tpool.tile([8, 17], f32r, tag="cr")
            nc.vector.tensor_copy(out=c_r[:], in_=c_tile[0:8, 0:17])
            diff = tpool.tile([M, BC], f32, tag="diff")
            nc.vector.tensor_sub(out=diff[:], in0=sw[:], in1=sb[:])
            sdiff = tpool.tile([M, BC], f32, tag="sdiff")
            nc.vector.tensor_mul(out=sdiff[:], in0=diff[:], in1=s_tile[:])
            # acc[0:4] = b + stm*(w-b);  acc[4:8] = w - stm*(w-b)
            accA = tpool.tile([M, BC], f32, tag="accA")
            nc.vector.tensor_add(out=accA[:], in0=sb[:], in1=sdiff[:])
            accB = tpool.tile([M, BC], f32, tag="accB")
            nc.vector.tensor_sub(out=accB[:], in0=sw[:], in1=sdiff[:])
            cA = tpool.tile([M, BC], f32r, tag="cA")
            nc.vector.tensor_scalar(out=cA[:], in0=accA[:], scalar1=0.0,
                                    scalar2=1.0, op0=alu.max, op1=alu.min)
            cB = tpool.tile([M, BC], f32r, tag="cB")
            nc.vector.tensor_scalar(out=cB[:], in0=accB[:], scalar1=0.0,
                                    scalar2=1.0, op0=alu.max, op1=alu.min)
            # l1: out[n, b] = sum_c l1_w[n, c] acc8[c, b], contraction 4+4
            # fp32r: single-pass PE, ~2^-13 rel err (vs 4-pass true fp32)
            for h in range(NB):
                sl = slice(h * 512, (h + 1) * 512)
                nc.tensor.matmul(p1[:, sl], c_r[0:4, 0:8],
                                 cA[:, sl],
                                 start=True, stop=False)
                nc.tensor.matmul(p1[:, sl], c_r[0:4, 8:16],
                                 cB[:, sl],
                                 start=False, stop=True)
            l1x = tpool.tile([8, BC], f32, tag="l1x")
            nc.vector.tensor_scalar_add(out=l1x[:], in0=p1[:],
                                        scalar1=c_tile[0:8, 18:19])
            l1c = tpool.tile([8, BC], f32r, tag="l1c")
            nc.vector.tensor_scalar(out=l1c[:], in0=l1x[:], scalar1=0.0,
                                    scalar2=1.0, op0=alu.max, op1=alu.min)
            p2 = ppool.tile([1, BC], f32, tag="p2")
            for h in range(NB):
                sl = slice(h * 512, (h + 1) * 512)
                nc.tensor.matmul(p2[:, sl], c_r[0:8, 16:17],
                                 l1c[:, sl],
                                 start=True, stop=True)
            ot = tpool.tile([1, BC], f32, tag="ot")
            nc.vector.tensor_scalar_add(out=ot[:], in0=p2[:],
                                        scalar1=c_tile[0:1, 19:20])
            nc.sync.dma_start(out=out[:], in_=ot[:])

    nc.finalize()
    _nc_cache[key] = nc
    return nc


def _pack_w(ft_w):
    """wsb[p, 4t+m] = ft_w[m, k(t,p)], k(t,p) = c*CHUNK + J*p + j, t = c*J+j."""
    ftwT = np.ascontiguousarray(ft_w.T)  # [K, 4]
    return (ftwT.reshape(NCHUNK, 128, J, M)
            .transpose(1, 0, 2, 3).reshape(128, NT * M).astype(BF))


def _prep_inputs(white_features, black_features, stm, ft_w, ft_b, l1_w, l1_b,
                 l2_w, l2_b):
    white_features = np.asarray(white_features, np.float32)
    black_features = np.asarray(black_features, np.float32)
    stm = np.asarray(stm, np.float32)
    ft_w = np.asarray(ft_w, np.float32)
    ft_b = np.asarray(ft_b, np.float32)
    l1_w = np.asarray(l1_w, np.float32)
    l1_b = np.asarray(l1_b, np.float32)
    l2_w = np.asarray(l2_w, np.float32)
    l2_b = np.asarray(l2_b, np.float32)

    wsb = _pack_w(ft_w)

    consts = np.zeros((8, 20), np.float32)
    consts[0:4, 0:8] = l1_w[:, 0:4].T
    consts[0:4, 8:16] = l1_w[:, 4:8].T
    consts[0:8, 16] = l2_w[0, :]
    # feature offset fold: w.f = (w.g)/16 + 0.5*sum_k(w) + ft_b
    consts[0:4, 17] = (ft_b.astype(np.float64)
                       + 0.5 * ft_w.astype(np.float64).sum(axis=1)
                       ).astype(np.float32)
    consts[0:8, 18] = l1_b
    consts[0, 19] = l2_b[0]

    # quantize features once: g = (f - 0.5) * 16 -> e3m4
    def quant(x):
        out = np.empty(x.shape, E3)
        CH = 1024
        for i in range(0, x.shape[0], CH):
            out[i:i + CH] = ((x[i:i + CH] - 0.5) * FSCALE).astype(E3)
        return out

    gq = {"featw": quant(white_features), "featb": quant(black_features)}

    in_maps = []
    for c in range(N_CORES):
        sl = slice(c * BC, (c + 1) * BC)
        m = {"wsb": wsb, "consts": consts,
             "stm4": np.ascontiguousarray(
                 np.broadcast_to(stm[sl][None, :], (M, BC)))}
        for name, g in gq.items():
            # [n, k] -> [c, p, j, n] with k = c*CHUNK + p*J + j
            feats = np.empty((NCHUNK, 128, J, BC), E3)
            feats[...] = g[sl].reshape(BC, NCHUNK, 128, J).transpose(1, 2, 3, 0)
            m[name] = feats.reshape(NCHUNK, 128, HB)
        in_maps.append(m)
    return in_maps


def _run(in_maps, trace=False, **kw):
    nc = _build_nc()
    res = run_bass_kernel_spmd(nc, in_maps, core_ids=list(range(N_CORES)),
                               trace=trace, **kw)
    out = np.concatenate(
        [r["out"].reshape(BC, 1) for r in res.results], axis=0)
    return out, res


def kernel(**inputs):
    in_maps = _prep_inputs(**inputs)
    out, _ = _run(in_maps, trace=False)
    return out


# revision 10
# speedup vs baseline: 1.0103x; 1.0103x over previous
"""HalfKP NNUE feature-transformer + MLP head for 8 Trainium2 NeuronCores.

Strategy (data-parallel over batch, fp8 feature stream):
  - Each of the 8 cores gets B/8 = 1024 batch rows of white/black features.
  - Features are centered+scaled on host: g = (f - 0.5) * 16, quantized to
    fp8 e3m4 (1 byte/elem -> 4x less HBM traffic than fp32). The 0.5 offset
    folds into the ft bias: w.f = (w.g)/16 + 0.5*sum(w) + ft_b.
  - ft weights stay bf16 (PE supports mixed bf16 lhsT x fp8 rhs) -> weight
    quantization error is negligible; only the e3m4 feature error remains
    (max rel err ~6e-3 on the final output).
  - PE 4-way column tiling: col group g = (side, half) accumulates its own
    [4, 512] slice of one PSUM bank at partitions 32g..32g+4. The four
    matmuls per k-tile run concurrently in the 128x128 array, so PE time is
    well below the DMA time -> kernel is HBM-bandwidth bound (~414 GB/s
    burst on 16 SDMA engines).
  - Host pre-transposes each feature shard to [K, Bc] so the contraction dim
    (K = 40960) lands on SBUF partitions with fully contiguous 16 KB/
    partition DMA descriptors. White chunks issue on the SP HWDGE ring,
    black chunks on the ACT ring, with separate sems so white's matmuls and
    the white-side tail math run while black still streams.
  - Tail (stm blend + clips + l1/l2) runs on [<=8, 1024] tiles; the tiny
    l1/l2 matmuls use fp32r (single-pass PE) to shorten the serial tail.
"""

import numpy as np
import ml_dtypes

import concourse.bass as bass
import concourse.bacc as bacc_mod
import concourse.mybir as mybir
from concourse.tile import TileContext
from concourse.bass_utils import run_bass_kernel_spmd

N_CORES = 8
B = 8192
K = 40960
M = 4
BC = B // N_CORES        # 1024 batch rows per core
CHUNK = 1024             # feature (k) rows per DMA chunk
J = CHUNK // 128         # k-slices per chunk (16)
NCHUNK = K // CHUNK      # 20
NB = BC // 512           # halves (matmul free-dim limit is 512 fp32)
NT = K // 128            # total k-tiles (320)
HB = J * BC              # free extent per chunk (8192)

FSCALE = 16.0            # feature scale: g = (f - 0.5) * FSCALE
FEAT_BUFS = 8

E3 = ml_dtypes.float8_e3m4
BF = ml_dtypes.bfloat16

_nc_cache = {}


def _build_nc():
    key = (CHUNK, FEAT_BUFS)
    if key in _nc_cache:
        return _nc_cache[key]
    f32 = mybir.dt.float32
    f32r = mybir.dt.float32r
    bf16 = mybir.dt.bfloat16
    e3 = mybir.dt.float8e3
    alu = mybir.AluOpType
    nc = bacc_mod.Bacc(trn_type="TRN2")

    featw = nc.dram_tensor("featw", [NCHUNK, 128, HB], e3,
                           kind="ExternalInput")
    featb = nc.dram_tensor("featb", [NCHUNK, 128, HB], e3,
                           kind="ExternalInput")
    # weight pack: wsb[p, 4t+m] = ft_w[m, k(t,p)] (bf16)
    wsb = nc.dram_tensor("wsb", [128, NT * M], bf16, kind="ExternalInput")
    consts = nc.dram_tensor("consts", [8, 20], f32, kind="ExternalInput")
    stm4 = nc.dram_tensor("stm4", [M, BC], f32, kind="ExternalInput")
    out = nc.dram_tensor("out", [1, BC], f32, kind="ExternalOutput")

    with TileContext(nc) as tc:
        with (
            tc.tile_pool(name="const", bufs=1) as cpool,
            tc.tile_pool(name="feat", bufs=FEAT_BUFS) as fpool,
            tc.tile_pool(name="psum", bufs=1, space="PSUM") as ppool,
            tc.tile_pool(name="tail", bufs=1) as tpool,
        ):
            # weights/consts lead the ACT ring (tiny, unblock the PE
            # warmups fast); feature chunks follow: white on SP, black on ACT.
            w_tile = cpool.tile([128, NT * M], bf16, tag="w")
            nc.scalar.dma_start(out=w_tile[:], in_=wsb[:])
            c_tile = cpool.tile([8, 20], f32, tag="c")
            nc.scalar.dma_start(out=c_tile[:], in_=consts[:])
            s_tile = cpool.tile([M, BC], f32, tag="s")
            nc.scalar.dma_start(out=s_tile[:], in_=stm4[:])

            # segments: (chunk, j0, jn) — one full chunk each
            segs = [(c, 0, J) for c in range(NCHUNK)]

            def seg_dma(i):
                c, j0, jn = segs[i]
                sub = jn != J
                tg = "l" if sub else ""
                ftw = fpool.tile([128, jn * BC], e3, tag=f"fw{tg}",
                                 name=f"fw{c}_{j0}")
                ftb_ = fpool.tile([128, jn * BC], e3, tag=f"fb{tg}",
                                  name=f"fb{c}_{j0}")
                # alternate rings per segment to balance the two HWDGE queues
                ring_w = nc.sync if i % 2 == 0 else nc.scalar
                ring_b = nc.scalar if i % 2 == 0 else nc.sync
                ring_w.dma_start(out=ftw[:], in_=featw[c, :, j0 * BC:(j0 + jn) * BC])
                ring_b.dma_start(out=ftb_[:], in_=featb[c, :, j0 * BC:(j0 + jn) * BC])
                return ftw, ftb_

            fts = [seg_dma(i) for i in range(FEAT_BUFS - 1)]

            # accumulator bank: col group g=(side,half) owns ps[32g:32g+4]
            ps = ppool.tile([128, 512], f32, tag="acc", name="acc")
            p1 = ppool.tile([8, BC], f32, tag="p1")
            # Warmup matmuls: consume the w_tile/c_tile DMA deps on PE so no
            # later matmul needs two sem waits (one HW wait slot per inst).
            nc.tensor.matmul(ps[0:4, 0:4], w_tile[:, 0:4], w_tile[:, 0:4],
                             start=True, stop=True, skip_group_check=True)
            nc.tensor.matmul(p1[0:8, 0:8], c_tile[0:4, 0:8],
                             c_tile[0:4, 0:8], start=True, stop=True,
                             skip_group_check=True)

            for i in range(len(segs)):
                c, j0, jn = segs[i]
                ftw, ftb_ = fts[i] if i < FEAT_BUFS - 1 else seg_dma(i)
                for j in range(j0, j0 + jn):
                    t = c * J + j
                    wt = w_tile[:, M * t: M * (t + 1)]
                    for g in range(4):
                        s, h = g >> 1, g & 1
                        src = ftw if s == 0 else ftb_
                        off = (j - j0) * BC + h * 512
                        nc.tensor.matmul(
                            ps[32 * g: 32 * g + 4, :],
                            wt, src[:, off: off + 512],
                            start=(t == 0),
                            stop=(t == NT - 1),
                            tile_position=(0, 32 * g),
                            skip_group_check=True,
                        )

            # ---- tail: scale+bias, stm blend, clips, l1, l2 ----
            # white-side combine depends only on col groups 0/1 -> runs
            # while black still streams.
            ftb = c_tile[0:M, 17:18]
            sw = tpool.tile([M, BC], f32, tag="sw")
            sb = tpool.tile([M, BC], f32, tag="sb")
            for h in range(NB):
                sl = slice(h * 512, (h + 1) * 512)
                nc.vector.tensor_scalar(
                    out=sw[:, sl], in0=ps[32 * h: 32 * h + 4, :],
                    scalar1=1.0 / FSCALE, scalar2=ftb,
                    op0=alu.mult, op1=alu.add)
            for h in range(NB):
                sl = slice(h * 512, (h + 1) * 512)
                nc.vector.tensor_scalar(
                    out=sb[:, sl], in0=ps[64 + 32 * h: 68 + 32 * h, :],
                    scalar1=1.0 / FSCALE, scalar2=ftb,
                    op0=alu.mult, op1=alu.add)
            c_r = tpool.tile([8, 17], f32r, tag="cr")
            nc.vector.tensor_copy(out=c_r[:], in_=c_tile[0:8, 0:17])
            diff = tpool.tile([M, BC], f32, tag="diff")
            nc.vector.tensor_sub(out=diff[:], in0=sw[:], in1=sb[:])
            sdiff = tpool.tile([M, BC], f32, tag="sdiff")
            nc.vector.tensor_mul(out=sdiff[:], in0=diff[:], in1=s_tile[:])
            # acc[0:4] = b + stm*(w-b);  acc[4:8] = w - stm*(w-b)
            accA = tpool.tile([M, BC], f32, tag="accA")
            nc.vector.tensor_add(out=accA[:], in0=sb[:], in1=sdiff[:])
            accB = tpool.tile([M, BC], f32, tag="accB")
            nc.vector.tensor_sub(out=accB[:], in0=sw[:], in1=sdiff[:])
            cA = tpool.tile([M, BC], f32r, tag="cA")
            nc.vector.tensor_scalar(out=cA[:], in0=accA[:], scalar1=0.0,
                                    scalar2=1.0, op0=alu.max, op1=alu.min)
            cB = tpool.tile([M, BC], f32r, tag="cB")
            nc.vector.tensor_scalar(out=cB[:], in0=accB[:], scalar1=0.0,
                                    scalar2=1.0, op0=alu.max, op1=alu.min)
            # l1: out[n, b] = sum_c l1_w[n, c] acc8[c, b], contraction 4+4
            # fp32r: single-pass PE, ~2^-13 rel err (vs 4-pass true fp32)
            for h in range(NB):
                sl = slice(h * 512, (h + 1) * 512)
                nc.tensor.matmul(p1[:, sl], c_r[0:4, 0:8],
                                 cA[:, sl],
                                 start=True, stop=False)
                nc.tensor.matmul(p1[:, sl], c_r[0:4, 8:16],
                                 cB[:, sl],
                                 start=False, stop=True)
            l1x = tpool.tile([8, BC], f32, tag="l1x")
            nc.vector.tensor_scalar_add(out=l1x[:], in0=p1[:],
                                        scalar1=c_tile[0:8, 18:19])
            l1c = tpool.tile([8, BC], f32r, tag="l1c")
            nc.vector.tensor_scalar(out=l1c[:], in0=l1x[:], scalar1=0.0,
                                    scalar2=1.0, op0=alu.max, op1=alu.min)
            p2 = ppool.tile([1, BC], f32, tag="p2")
            for h in range(NB):
                sl = slice(h * 512, (h + 1) * 512)
                nc.tensor.matmul(p2[:, sl], c_r[0:8, 16:17],
                                 l1c[:, sl],
                                 start=True, stop=True)
            ot = tpool.tile([1, BC], f32, tag="ot")
            nc.vector.tensor_scalar_add(out=ot[:], in0=p2[:],
                                        scalar1=c_tile[0:1, 19:20])
            nc.sync.dma_start(out=out[:], in_=ot[:])

    nc.finalize()
    _nc_cache[key] = nc
    return nc


def _pack_w(ft_w):
    """wsb[p, 4t+m] = ft_w[m, k(t,p)], k(t,p) = c*CHUNK + J*p + j, t = c*J+j."""
    ftwT = np.ascontiguousarray(ft_w.T)  # [K, 4]
    return (ftwT.reshape(NCHUNK, 128, J, M)
            .transpose(1, 0, 2, 3).reshape(128, NT * M).astype(BF))


def _prep_inputs(white_features, black_features, stm, ft_w, ft_b, l1_w, l1_b,
                 l2_w, l2_b):
    white_features = np.asarray(white_features, np.float32)
    black_features = np.asarray(black_features, np.float32)
    stm = np.asarray(stm, np.float32)
    ft_w = np.asarray(ft_w, np.float32)
    ft_b = np.asarray(ft_b, np.float32)
    l1_w = np.asarray(l1_w, np.float32)
    l1_b = np.asarray(l1_b, np.float32)
    l2_w = np.asarray(l2_w, np.float32)
    l2_b = np.asarray(l2_b, np.float32)

    wsb = _pack_w(ft_w)

    consts = np.zeros((8, 20), np.float32)
    consts[0:4, 0:8] = l1_w[:, 0:4].T
    consts[0:4, 8:16] = l1_w[:, 4:8].T
    consts[0:8, 16] = l2_w[0, :]
    # feature offset fold: w.f = (w.g)/16 + 0.5*sum_k(w) + ft_b
    consts[0:4, 17] = (ft_b.astype(np.float64)
                       + 0.5 * ft_w.astype(np.float64).sum(axis=1)
                       ).astype(np.float32)
    consts[0:8, 18] = l1_b
    consts[0, 19] = l2_b[0]

    # quantize features once: g = (f - 0.5) * 16 -> e3m4
    def quant(x):
        out = np.empty(x.shape, E3)
        CH = 1024
        for i in range(0, x.shape[0], CH):
            out[i:i + CH] = ((x[i:i + CH] - 0.5) * FSCALE).astype(E3)
        return out

    gq = {"featw": quant(white_features), "featb": quant(black_features)}

    in_maps = []
    for c in range(N_CORES):
        sl = slice(c * BC, (c + 1) * BC)
        m = {"wsb": wsb, "consts": consts,
             "stm4": np.ascontiguousarray(
                 np.broadcast_to(stm[sl][None, :], (M, BC)))}
        for name, g in gq.items():
            # [n, k] -> [c, p, j, n] with k = c*CHUNK + p*J + j
            feats = np.empty((NCHUNK, 128, J, BC), E3)
            feats[...] = g[sl].reshape(BC, NCHUNK, 128, J).transpose(1, 2, 3, 0)
            m[name] = feats.reshape(NCHUNK, 128, HB)
        in_maps.append(m)
    return in_maps


def _run(in_maps, trace=False, **kw):
    nc = _build_nc()
    res = run_bass_kernel_spmd(nc, in_maps, core_ids=list(range(N_CORES)),
                               trace=trace, **kw)
    out = np.concatenate(
        [r["out"].reshape(BC, 1) for r in res.results], axis=0)
    return out, res


def kernel(**inputs):
    in_maps = _prep_inputs(**inputs)
    out, _ = _run(in_maps, trace=False)
    return out


# revision 11
# speedup vs baseline: 1.1046x; 1.0933x over previous
"""HalfKP NNUE feature-transformer + MLP head for 8 Trainium2 NeuronCores.

Strategy (data-parallel over batch, fp8 feature stream):
  - Each of the 8 cores gets B/8 = 1024 batch rows of white/black features.
  - Features are centered+scaled on host: g = (f - 0.5) * 16, quantized to
    fp8 e3m4 (1 byte/elem -> 4x less HBM traffic than fp32). The 0.5 offset
    folds into the ft bias: w.f = (w.g)/16 + 0.5*sum(w) + ft_b.
  - ft weights stay bf16 (PE supports mixed bf16 lhsT x fp8 rhs) -> weight
    quantization error is negligible; only the e3m4 feature error remains
    (max rel err ~6e-3 on the final output).
  - PE 4-way column tiling: col group g = (side, half) accumulates its own
    [4, 512] slice of one PSUM bank at partitions 32g..32g+4. The four
    matmuls per k-tile run concurrently in the 128x128 array, so PE time is
    well below the DMA time -> kernel is HBM-bandwidth bound (~414 GB/s
    burst on 16 SDMA engines).
  - Host pre-transposes each feature shard to [K, Bc] so the contraction dim
    (K = 40960) lands on SBUF partitions with fully contiguous 16 KB/
    partition DMA descriptors. White chunks issue on the SP HWDGE ring,
    black chunks on the ACT ring, with separate sems so white's matmuls and
    the white-side tail math run while black still streams.
  - Tail (stm blend + clips + l1/l2) runs on [<=8, 1024] tiles; the tiny
    l1/l2 matmuls use fp32r (single-pass PE) to shorten the serial tail.
"""

import numpy as np
import ml_dtypes

import concourse.bass as bass
import concourse.bacc as bacc_mod
import concourse.mybir as mybir
from concourse.tile import TileContext
from concourse.bass_utils import run_bass_kernel_spmd

N_CORES = 8
B = 8192
K = 40960
M = 4
BC = B // N_CORES        # 1024 batch rows per core
CHUNK = 2048             # feature (k) rows per DMA chunk
J = CHUNK // 128         # k-slices per chunk (16)
NCHUNK = K // CHUNK      # 20
NB = BC // 512           # halves (matmul free-dim limit is 512 fp32)
NT = K // 128            # total k-tiles (320)
HB = J * BC              # free extent per chunk (8192)

FSCALE = 16.0            # feature scale: g = (f - 0.5) * FSCALE
FEAT_BUFS = 4

E3 = ml_dtypes.float8_e3m4
BF = ml_dtypes.bfloat16

_nc_cache = {}


def _build_nc():
    key = (CHUNK, FEAT_BUFS)
    if key in _nc_cache:
        return _nc_cache[key]
    f32 = mybir.dt.float32
    f32r = mybir.dt.float32r
    bf16 = mybir.dt.bfloat16
    e3 = mybir.dt.float8e3
    alu = mybir.AluOpType
    nc = bacc_mod.Bacc(trn_type="TRN2")

    featw = nc.dram_tensor("featw", [NCHUNK, 128, HB], e3,
                           kind="ExternalInput")
    featb = nc.dram_tensor("featb", [NCHUNK, 128, HB], e3,
                           kind="ExternalInput")
    # weight pack: wsb[p, 4t+m] = ft_w[m, k(t,p)] (bf16)
    wsb = nc.dram_tensor("wsb", [128, NT * M], bf16, kind="ExternalInput")
    consts = nc.dram_tensor("consts", [8, 20], f32, kind="ExternalInput")
    stm4 = nc.dram_tensor("stm4", [M, BC], f32, kind="ExternalInput")
    out = nc.dram_tensor("out", [1, BC], f32, kind="ExternalOutput")

    with TileContext(nc) as tc:
        with (
            tc.tile_pool(name="const", bufs=1) as cpool,
            tc.tile_pool(name="feat", bufs=FEAT_BUFS) as fpool,
            tc.tile_pool(name="psum", bufs=1, space="PSUM") as ppool,
            tc.tile_pool(name="tail", bufs=1) as tpool,
        ):
            # weights/consts lead the ACT ring (tiny, unblock the PE
            # warmups fast); feature chunks follow: white on SP, black on ACT.
            w_tile = cpool.tile([128, NT * M], bf16, tag="w")
            nc.scalar.dma_start(out=w_tile[:], in_=wsb[:])
            c_tile = cpool.tile([8, 20], f32, tag="c")
            nc.scalar.dma_start(out=c_tile[:], in_=consts[:])
            s_tile = cpool.tile([M, BC], f32, tag="s")
            nc.scalar.dma_start(out=s_tile[:], in_=stm4[:])

            # segments: (chunk, j0, jn) — one full chunk each
            segs = [(c, 0, J) for c in range(NCHUNK)]

            def seg_dma(i):
                c, j0, jn = segs[i]
                sub = jn != J
                tg = "l" if sub else ""
                ftw = fpool.tile([128, jn * BC], e3, tag=f"fw{tg}",
                                 name=f"fw{c}_{j0}")
                ftb_ = fpool.tile([128, jn * BC], e3, tag=f"fb{tg}",
                                  name=f"fb{c}_{j0}")
                ring_w = nc.sync
                ring_b = nc.scalar
                ring_w.dma_start(out=ftw[:], in_=featw[c, :, j0 * BC:(j0 + jn) * BC])
                ring_b.dma_start(out=ftb_[:], in_=featb[c, :, j0 * BC:(j0 + jn) * BC])
                return ftw, ftb_

            fts = [seg_dma(i) for i in range(FEAT_BUFS - 1)]

            # accumulator bank: col group g=(side,half) owns ps[32g:32g+4]
            ps = ppool.tile([128, 512], f32, tag="acc", name="acc")
            p1 = ppool.tile([8, BC], f32, tag="p1")
            # Warmup matmuls: consume the w_tile/c_tile DMA deps on PE so no
            # later matmul needs two sem waits (one HW wait slot per inst).
            nc.tensor.matmul(ps[0:4, 0:4], w_tile[:, 0:4], w_tile[:, 0:4],
                             start=True, stop=True, skip_group_check=True)
            nc.tensor.matmul(p1[0:8, 0:8], c_tile[0:4, 0:8],
                             c_tile[0:4, 0:8], start=True, stop=True,
                             skip_group_check=True)

            for i in range(len(segs)):
                c, j0, jn = segs[i]
                ftw, ftb_ = fts[i] if i < FEAT_BUFS - 1 else seg_dma(i)
                for j in range(j0, j0 + jn):
                    t = c * J + j
                    wt = w_tile[:, M * t: M * (t + 1)]
                    for g in range(4):
                        s, h = g >> 1, g & 1
                        src = ftw if s == 0 else ftb_
                        off = (j - j0) * BC + h * 512
                        nc.tensor.matmul(
                            ps[32 * g: 32 * g + 4, :],
                            wt, src[:, off: off + 512],
                            start=(t == 0),
                            stop=(t == NT - 1),
                            tile_position=(0, 32 * g),
                            skip_group_check=True,
                        )

            # ---- tail: scale+bias, stm blend, clips, l1, l2 ----
            # white-side combine depends only on col groups 0/1 -> runs
            # while black still streams.
            ftb = c_tile[0:M, 17:18]
            sw = tpool.tile([M, BC], f32, tag="sw")
            sb = tpool.tile([M, BC], f32, tag="sb")
            for h in range(NB):
                sl = slice(h * 512, (h + 1) * 512)
                nc.vector.tensor_scalar(
                    out=sw[:, sl], in0=ps[32 * h: 32 * h + 4, :],
                    scalar1=1.0 / FSCALE, scalar2=ftb,
                    op0=alu.mult, op1=alu.add)
            for h in range(NB):
                sl = slice(h * 512, (h + 1) * 512)
                nc.vector.tensor_scalar(
                    out=sb[:, sl], in0=ps[64 + 32 * h: 68 + 32 * h, :],
                    scalar1=1.0 / FSCALE, scalar2=ftb,
                    op0=alu.mult, op1=alu.add)
            c_r = tpool.tile([8, 17], f32r, tag="cr")
            nc.vector.tensor_copy(out=c_r[:], in_=c_tile[0:8, 0:17])
            diff = tpool.tile([M, BC], f32, tag="diff")
            nc.vector.tensor_sub(out=diff[:], in0=sw[:], in1=sb[:])
            sdiff = tpool.tile([M, BC], f32, tag="sdiff")
            nc.vector.tensor_mul(out=sdiff[:], in0=diff[:], in1=s_tile[:])
            # acc[0:4] = b + stm*(w-b);  acc[4:8] = w - stm*(w-b)
            accA = tpool.tile([M, BC], f32, tag="accA")
            nc.vector.tensor_add(out=accA[:], in0=sb[:], in1=sdiff[:])
            accB = tpool.tile([M, BC], f32, tag="accB")
            nc.vector.tensor_sub(out=accB[:], in0=sw[:], in1=sdiff[:])
            cA = tpool.tile([M, BC], f32r, tag="cA")
            nc.vector.tensor_scalar(out=cA[:], in0=accA[:], scalar1=0.0,
                                    scalar2=1.0, op0=alu.max, op1=alu.min)
            cB = tpool.tile([M, BC], f32r, tag="cB")
            nc.vector.tensor_scalar(out=cB[:], in0=accB[:], scalar1=0.0,
                                    scalar2=1.0, op0=alu.max, op1=alu.min)
            # l1: out[n, b] = sum_c l1_w[n, c] acc8[c, b], contraction 4+4
            # fp32r: single-pass PE, ~2^-13 rel err (vs 4-pass true fp32)
            for h in range(NB):
                sl = slice(h * 512, (h + 1) * 512)
                nc.tensor.matmul(p1[:, sl], c_r[0:4, 0:8],
                                 cA[:, sl],
                                 start=True, stop=False)
                nc.tensor.matmul(p1[:, sl], c_r[0:4, 8:16],
                                 cB[:, sl],
                                 start=False, stop=True)
            l1x = tpool.tile([8, BC], f32, tag="l1x")
            nc.vector.tensor_scalar_add(out=l1x[:], in0=p1[:],
                                        scalar1=c_tile[0:8, 18:19])
            l1c = tpool.tile([8, BC], f32r, tag="l1c")
            nc.vector.tensor_scalar(out=l1c[:], in0=l1x[:], scalar1=0.0,
                                    scalar2=1.0, op0=alu.max, op1=alu.min)
            p2 = ppool.tile([1, BC], f32, tag="p2")
            for h in range(NB):
                sl = slice(h * 512, (h + 1) * 512)
                nc.tensor.matmul(p2[:, sl], c_r[0:8, 16:17],
                                 l1c[:, sl],
                                 start=True, stop=True)
            ot = tpool.tile([1, BC], f32, tag="ot")
            nc.vector.tensor_scalar_add(out=ot[:], in0=p2[:],
                                        scalar1=c_tile[0:1, 19:20])
            nc.sync.dma_start(out=out[:], in_=ot[:])

    nc.finalize()
    _nc_cache[key] = nc
    return nc


def _pack_w(ft_w):
    """wsb[p, 4t+m] = ft_w[m, k(t,p)], k(t,p) = c*CHUNK + J*p + j, t = c*J+j."""
    ftwT = np.ascontiguousarray(ft_w.T)  # [K, 4]
    return (ftwT.reshape(NCHUNK, 128, J, M)
            .transpose(1, 0, 2, 3).reshape(128, NT * M).astype(BF))


def _prep_inputs(white_features, black_features, stm, ft_w, ft_b, l1_w, l1_b,
                 l2_w, l2_b):
    white_features = np.asarray(white_features, np.float32)
    black_features = np.asarray(black_features, np.float32)
    stm = np.asarray(stm, np.float32)
    ft_w = np.asarray(ft_w, np.float32)
    ft_b = np.asarray(ft_b, np.float32)
    l1_w = np.asarray(l1_w, np.float32)
    l1_b = np.asarray(l1_b, np.float32)
    l2_w = np.asarray(l2_w, np.float32)
    l2_b = np.asarray(l2_b, np.float32)

    wsb = _pack_w(ft_w)

    consts = np.zeros((8, 20), np.float32)
    consts[0:4, 0:8] = l1_w[:, 0:4].T
    consts[0:4, 8:16] = l1_w[:, 4:8].T
    consts[0:8, 16] = l2_w[0, :]
    # feature offset fold: w.f = (w.g)/16 + 0.5*sum_k(w) + ft_b
    consts[0:4, 17] = (ft_b.astype(np.float64)
                       + 0.5 * ft_w.astype(np.float64).sum(axis=1)
                       ).astype(np.float32)
    consts[0:8, 18] = l1_b
    consts[0, 19] = l2_b[0]

    # quantize features once: g = (f - 0.5) * 16 -> e3m4
    def quant(x):
        out = np.empty(x.shape, E3)
        CH = 1024
        for i in range(0, x.shape[0], CH):
            out[i:i + CH] = ((x[i:i + CH] - 0.5) * FSCALE).astype(E3)
        return out

    gq = {"featw": quant(white_features), "featb": quant(black_features)}

    in_maps = []
    for c in range(N_CORES):
        sl = slice(c * BC, (c + 1) * BC)
        m = {"wsb": wsb, "consts": consts,
             "stm4": np.ascontiguousarray(
                 np.broadcast_to(stm[sl][None, :], (M, BC)))}
        for name, g in gq.items():
            # [n, k] -> [c, p, j, n] with k = c*CHUNK + p*J + j
            feats = np.empty((NCHUNK, 128, J, BC), E3)
            feats[...] = g[sl].reshape(BC, NCHUNK, 128, J).transpose(1, 2, 3, 0)
            m[name] = feats.reshape(NCHUNK, 128, HB)
        in_maps.append(m)
    return in_maps


def _run(in_maps, trace=False, **kw):
    nc = _build_nc()
    res = run_bass_kernel_spmd(nc, in_maps, core_ids=list(range(N_CORES)),
                               trace=trace, **kw)
    out = np.concatenate(
        [r["out"].reshape(BC, 1) for r in res.results], axis=0)
    return out, res


def kernel(**inputs):
    in_maps = _prep_inputs(**inputs)
    out, _ = _run(in_maps, trace=False)
    return out


# revision 12
# speedup vs baseline: 1.1418x; 1.0337x over previous
"""HalfKP NNUE feature-transformer + MLP head for 8 Trainium2 NeuronCores.

Strategy (data-parallel over batch, fp8 feature stream):
  - Each of the 8 cores gets B/8 = 1024 batch rows of white/black features.
  - Features are centered+scaled on host: g = (f - 0.5) * 16, quantized to
    fp8 e3m4 (1 byte/elem -> 4x less HBM traffic than fp32). The 0.5 offset
    folds into the ft bias: w.f = (w.g)/16 + 0.5*sum(w) + ft_b.
  - ft weights stay bf16 (PE supports mixed bf16 lhsT x fp8 rhs) -> weight
    quantization error is negligible; only the e3m4 feature error remains
    (max rel err ~6e-3 on the final output).
  - PE 4-way column tiling: col group g = (side, half) accumulates its own
    [4, 512] slice of one PSUM bank at partitions 32g..32g+4. The four
    matmuls per k-tile run concurrently in the 128x128 array, so PE time is
    well below the DMA time -> kernel is HBM-bandwidth bound (~414 GB/s
    burst on 16 SDMA engines).
  - Host pre-transposes each feature shard to [K, Bc] so the contraction dim
    (K = 40960) lands on SBUF partitions with fully contiguous 16 KB/
    partition DMA descriptors. White chunks issue on the SP HWDGE ring,
    black chunks on the ACT ring, with separate sems so white's matmuls and
    the white-side tail math run while black still streams.
  - Tail (stm blend + clips + l1/l2) runs on [<=8, 1024] tiles; the tiny
    l1/l2 matmuls use fp32r (single-pass PE) to shorten the serial tail.
"""

import numpy as np
import ml_dtypes

import concourse.bass as bass
import concourse.bacc as bacc_mod
import concourse.mybir as mybir
from concourse.tile import TileContext
from concourse.bass_utils import run_bass_kernel_spmd

N_CORES = 8
B = 8192
K = 40960
M = 4
BC = B // N_CORES        # 1024 batch rows per core
CHUNK = 1024             # feature (k) rows per DMA chunk
J = CHUNK // 128         # k-slices per chunk (16)
NCHUNK = K // CHUNK      # 20
NB = BC // 512           # halves (matmul free-dim limit is 512 fp32)
NT = K // 128            # total k-tiles (320)
HB = J * BC              # free extent per chunk (8192)

FSCALE = 16.0            # feature scale: g = (f - 0.5) * FSCALE
FEAT_BUFS = 8

E3 = ml_dtypes.float8_e3m4
BF = ml_dtypes.bfloat16

_nc_cache = {}


def _build_nc():
    key = (CHUNK, FEAT_BUFS)
    if key in _nc_cache:
        return _nc_cache[key]
    f32 = mybir.dt.float32
    f32r = mybir.dt.float32r
    bf16 = mybir.dt.bfloat16
    e3 = mybir.dt.float8e3
    alu = mybir.AluOpType
    nc = bacc_mod.Bacc(trn_type="TRN2")

    feats = nc.dram_tensor("feats", [NCHUNK, 128, 2 * HB], e3,
                           kind="ExternalInput")
    # weight pack: wsb[p, 4t+m] = ft_w[m, k(t,p)] (bf16)
    wsb = nc.dram_tensor("wsb", [128, NT * M], bf16, kind="ExternalInput")
    consts = nc.dram_tensor("consts", [8, 20], f32, kind="ExternalInput")
    stm4 = nc.dram_tensor("stm4", [M, BC], f32, kind="ExternalInput")
    out = nc.dram_tensor("out", [1, BC], f32, kind="ExternalOutput")

    with TileContext(nc) as tc:
        with (
            tc.tile_pool(name="const", bufs=1) as cpool,
            tc.tile_pool(name="feat", bufs=FEAT_BUFS) as fpool,
            tc.tile_pool(name="psum", bufs=1, space="PSUM") as ppool,
            tc.tile_pool(name="tail", bufs=1) as tpool,
        ):
            # weights/consts lead the ACT ring (tiny, unblock the PE
            # warmups fast); feature chunks follow: white on SP, black on ACT.
            w_tile = cpool.tile([128, NT * M], bf16, tag="w")
            nc.scalar.dma_start(out=w_tile[:], in_=wsb[:])
            c_tile = cpool.tile([8, 20], f32, tag="c")
            nc.scalar.dma_start(out=c_tile[:], in_=consts[:])
            s_tile = cpool.tile([M, BC], f32, tag="s")
            nc.scalar.dma_start(out=s_tile[:], in_=stm4[:])

            # segments: (chunk, j0, jn) — one full chunk each
            segs = [(c, 0, J) for c in range(NCHUNK)]

            def seg_dma(i):
                c, j0, jn = segs[i]
                ft = fpool.tile([128, 2 * HB], e3, tag="ft", name=f"ft{c}")
                nc.sync.dma_start(out=ft[:], in_=feats[c])
                return ft

            fts = [seg_dma(i) for i in range(FEAT_BUFS - 1)]

            # accumulator bank: col group g=(side,half) owns ps[32g:32g+4]
            ps = ppool.tile([128, 512], f32, tag="acc", name="acc")
            p1 = ppool.tile([8, BC], f32, tag="p1")
            # Warmup matmuls: consume the w_tile/c_tile DMA deps on PE so no
            # later matmul needs two sem waits (one HW wait slot per inst).
            nc.tensor.matmul(ps[0:4, 0:4], w_tile[:, 0:4], w_tile[:, 0:4],
                             start=True, stop=True, skip_group_check=True)
            nc.tensor.matmul(p1[0:8, 0:8], c_tile[0:4, 0:8],
                             c_tile[0:4, 0:8], start=True, stop=True,
                             skip_group_check=True)

            for i in range(len(segs)):
                c, j0, jn = segs[i]
                ft = fts[i] if i < FEAT_BUFS - 1 else seg_dma(i)
                for j in range(j0, j0 + jn):
                    t = c * J + j
                    wt = w_tile[:, M * t: M * (t + 1)]
                    for g in range(4):
                        s, h = g >> 1, g & 1
                        off = s * HB + j * BC + h * 512
                        nc.tensor.matmul(
                            ps[32 * g: 32 * g + 4, :],
                            wt, ft[:, off: off + 512],
                            start=(t == 0),
                            stop=(t == NT - 1),
                            tile_position=(0, 32 * g),
                            skip_group_check=True,
                        )

            # ---- tail: scale+bias, stm blend, clips, l1, l2 ----
            # white-side combine depends only on col groups 0/1 -> runs
            # while black still streams.
            ftb = c_tile[0:M, 17:18]
            sw = tpool.tile([M, BC], f32, tag="sw")
            sb = tpool.tile([M, BC], f32, tag="sb")
            for h in range(NB):
                sl = slice(h * 512, (h + 1) * 512)
                nc.vector.tensor_scalar(
                    out=sw[:, sl], in0=ps[32 * h: 32 * h + 4, :],
                    scalar1=1.0 / FSCALE, scalar2=ftb,
                    op0=alu.mult, op1=alu.add)
            for h in range(NB):
                sl = slice(h * 512, (h + 1) * 512)
                nc.vector.tensor_scalar(
                    out=sb[:, sl], in0=ps[64 + 32 * h: 68 + 32 * h, :],
                    scalar1=1.0 / FSCALE, scalar2=ftb,
                    op0=alu.mult, op1=alu.add)
            c_r = tpool.tile([8, 17], f32r, tag="cr")
            nc.vector.tensor_copy(out=c_r[:], in_=c_tile[0:8, 0:17])
            diff = tpool.tile([M, BC], f32, tag="diff")
            nc.vector.tensor_sub(out=diff[:], in0=sw[:], in1=sb[:])
            sdiff = tpool.tile([M, BC], f32, tag="sdiff")
            nc.vector.tensor_mul(out=sdiff[:], in0=diff[:], in1=s_tile[:])
            # acc[0:4] = b + stm*(w-b);  acc[4:8] = w - stm*(w-b)
            accA = tpool.tile([M, BC], f32, tag="accA")
            nc.vector.tensor_add(out=accA[:], in0=sb[:], in1=sdiff[:])
            accB = tpool.tile([M, BC], f32, tag="accB")
            nc.vector.tensor_sub(out=accB[:], in0=sw[:], in1=sdiff[:])
            cA = tpool.tile([M, BC], f32r, tag="cA")
            nc.vector.tensor_scalar(out=cA[:], in0=accA[:], scalar1=0.0,
                                    scalar2=1.0, op0=alu.max, op1=alu.min)
            cB = tpool.tile([M, BC], f32r, tag="cB")
            nc.vector.tensor_scalar(out=cB[:], in0=accB[:], scalar1=0.0,
                                    scalar2=1.0, op0=alu.max, op1=alu.min)
            # l1: out[n, b] = sum_c l1_w[n, c] acc8[c, b], contraction 4+4
            # fp32r: single-pass PE, ~2^-13 rel err (vs 4-pass true fp32)
            for h in range(NB):
                sl = slice(h * 512, (h + 1) * 512)
                nc.tensor.matmul(p1[:, sl], c_r[0:4, 0:8],
                                 cA[:, sl],
                                 start=True, stop=False)
                nc.tensor.matmul(p1[:, sl], c_r[0:4, 8:16],
                                 cB[:, sl],
                                 start=False, stop=True)
            l1x = tpool.tile([8, BC], f32, tag="l1x")
            nc.vector.tensor_scalar_add(out=l1x[:], in0=p1[:],
                                        scalar1=c_tile[0:8, 18:19])
            l1c = tpool.tile([8, BC], f32r, tag="l1c")
            nc.vector.tensor_scalar(out=l1c[:], in0=l1x[:], scalar1=0.0,
                                    scalar2=1.0, op0=alu.max, op1=alu.min)
            p2 = ppool.tile([1, BC], f32, tag="p2")
            for h in range(NB):
                sl = slice(h * 512, (h + 1) * 512)
                nc.tensor.matmul(p2[:, sl], c_r[0:8, 16:17],
                                 l1c[:, sl],
                                 start=True, stop=True)
            ot = tpool.tile([1, BC], f32, tag="ot")
            nc.vector.tensor_scalar_add(out=ot[:], in0=p2[:],
                                        scalar1=c_tile[0:1, 19:20])
            nc.sync.dma_start(out=out[:], in_=ot[:])

    nc.finalize()
    _nc_cache[key] = nc
    return nc


def _pack_w(ft_w):
    """wsb[p, 4t+m] = ft_w[m, k(t,p)], k(t,p) = c*CHUNK + J*p + j, t = c*J+j."""
    ftwT = np.ascontiguousarray(ft_w.T)  # [K, 4]
    return (ftwT.reshape(NCHUNK, 128, J, M)
            .transpose(1, 0, 2, 3).reshape(128, NT * M).astype(BF))


def _prep_inputs(white_features, black_features, stm, ft_w, ft_b, l1_w, l1_b,
                 l2_w, l2_b):
    white_features = np.asarray(white_features, np.float32)
    black_features = np.asarray(black_features, np.float32)
    stm = np.asarray(stm, np.float32)
    ft_w = np.asarray(ft_w, np.float32)
    ft_b = np.asarray(ft_b, np.float32)
    l1_w = np.asarray(l1_w, np.float32)
    l1_b = np.asarray(l1_b, np.float32)
    l2_w = np.asarray(l2_w, np.float32)
    l2_b = np.asarray(l2_b, np.float32)

    wsb = _pack_w(ft_w)

    consts = np.zeros((8, 20), np.float32)
    consts[0:4, 0:8] = l1_w[:, 0:4].T
    consts[0:4, 8:16] = l1_w[:, 4:8].T
    consts[0:8, 16] = l2_w[0, :]
    # feature offset fold: w.f = (w.g)/16 + 0.5*sum_k(w) + ft_b
    consts[0:4, 17] = (ft_b.astype(np.float64)
                       + 0.5 * ft_w.astype(np.float64).sum(axis=1)
                       ).astype(np.float32)
    consts[0:8, 18] = l1_b
    consts[0, 19] = l2_b[0]

    # quantize features once: g = (f - 0.5) * 16 -> e3m4
    def quant(x):
        out = np.empty(x.shape, E3)
        CH = 1024
        for i in range(0, x.shape[0], CH):
            out[i:i + CH] = ((x[i:i + CH] - 0.5) * FSCALE).astype(E3)
        return out

    gw = quant(white_features)
    gb = quant(black_features)

    in_maps = []
    for c in range(N_CORES):
        sl = slice(c * BC, (c + 1) * BC)
        feats = np.empty((NCHUNK, 128, 2, J, BC), E3)
        for s, g in enumerate((gw, gb)):
            # [n, k] -> [c, p, j, n] with k = c*CHUNK + p*J + j
            feats[:, :, s] = g[sl].reshape(BC, NCHUNK, 128, J).transpose(1, 2, 3, 0)
        m = {"feats": feats.reshape(NCHUNK, 128, 2 * HB),
             "wsb": wsb, "consts": consts,
             "stm4": np.ascontiguousarray(
                 np.broadcast_to(stm[sl][None, :], (M, BC)))}
        in_maps.append(m)
    return in_maps


def _run(in_maps, trace=False, **kw):
    nc = _build_nc()
    res = run_bass_kernel_spmd(nc, in_maps, core_ids=list(range(N_CORES)),
                               trace=trace, **kw)
    out = np.concatenate(
        [r["out"].reshape(BC, 1) for r in res.results], axis=0)
    return out, res


def kernel(**inputs):
    in_maps = _prep_inputs(**inputs)
    out, _ = _run(in_maps, trace=False)
    return out
